# revision 11
# baseline (speedup 1.0000x reference)
"""Trainium2 Bass kernel for a 1-layer transformer encoder.

Reference model (B=32, S=512, D=768, H=12, hd=64, hidden=3072):
    q,k,v = x@Wq, x@Wk, x@Wv         (per head)
    attn  = softmax(q k^T / 8) v
    mha   = concat_heads @ Wo
    out1  = x + LN(mha)
    ffn   = gelu(out1@W1 + b1) @ W2 + b2
    out   = out1 + LN(ffn)

Sharding: data-parallel over batch -- each of the 8 cores gets 4 full
sequences (2048 tokens) and all weights; no collectives.

Key kernel tricks:
  * All matmuls in bf16 (4x faster PE than fp32), fp32 PSUM accumulation.
  * Softmax computed as scores^T (t on partitions): exp without max
    subtraction (scores are O(1) here by construction) and WITHOUT the
    denominator: LN(c*v) == LN(v) for any per-token c>0, and the
    attention output only feeds Wo -> LN, so softmax normalization
    cancels mathematically.
  * LN rstd via ACT Sqrt + DVE reciprocal (production groupnorm idiom).
  * Per-feature LN scale/shift DMA-broadcast along partitions; b2 added
    via a rank-1 (ones x b2) matmul accumulated into PSUM.
  * Activations stay "feature-transposed" exactly where matmuls want
    them; only x and out1 need explicit PE transposes (96 blocks each).
"""

import os
import sys
from contextlib import ExitStack

import numpy as np

for _p in ("/opt/trn_rl_repo", "/root/.axon_site/_ro/trn_rl_repo"):
    if os.path.isdir(_p) and _p not in sys.path:
        sys.path.insert(0, _p)

os.environ.setdefault("MYCRO_LOCAL_CACHE", "1")

import concourse.bacc as bacc
import concourse.tile as tile
from concourse import mybir
from concourse.bass_utils import run_bass_kernel_spmd
from concourse.masks import make_identity

F32 = mybir.dt.float32
BF16 = mybir.dt.bfloat16
AF = mybir.ActivationFunctionType
OP = mybir.AluOpType

# model dims
D, NHEAD, HD, FF, SEQ, P = 768, 12, 64, 3072, 512, 128
ND = D // P    # 6 feature chunks
NF = FF // P   # 24 hidden chunks
EPS = 1e-5
N_CORES = 8
B_TOTAL = 32


def _ln_apply(nc, pool, src_psum, w_bc, b_plus_res, out_tile, tag, eps_tile):
    """out_tile = b_plus_res + ((src - mean)/std) * w_bc   (src: [128, 768] psum)."""
    stats = pool.tile([P, 3, 6], F32, tag=f"stats{tag}")
    for sg in range(3):
        nc.vector.bn_stats(out=stats[:, sg, :], in_=src_psum[:, sg * 256:(sg + 1) * 256])
    mv = pool.tile([P, 2], F32, tag=f"mv{tag}")
    nc.vector.bn_aggr(out=mv, in_=stats)
    rstd = pool.tile([P, 1], F32, tag=f"rstd{tag}")
    # rstd = 1/sqrt(var + eps)
    nc.scalar.activation(out=rstd, in_=mv[:, 1:2], func=AF.Sqrt, bias=eps_tile, scale=1.0)
    nc.vector.reciprocal(out=rstd, in_=rstd)
    # u = (src - mean) * w_bc   [one STT pass from PSUM]
    u = pool.tile([P, D], F32, tag=f"u{tag}")
    nc.vector.scalar_tensor_tensor(
        out=u, in0=src_psum, scalar=mv[:, 0:1], in1=w_bc,
        op0=OP.subtract, op1=OP.mult,
    )
    # out = u * rstd + b_plus_res
    nc.vector.scalar_tensor_tensor(
        out=out_tile, in0=u, scalar=rstd, in1=b_plus_res,
        op0=OP.mult, op1=OP.add,
    )


def emit(nc, tc, io, bpc):
    """Emit the whole encoder layer for one core holding `bpc` sequences."""
    T = bpc * SEQ
    NT = T // P          # token chunks
    NB = bpc             # 512-wide column groups == batches

    # Pools are opened/released manually: SBUF reservations are
    # [open, release] windows, and the phase structure is not LIFO.
    consts = tc.alloc_tile_pool(name="consts", bufs=1)
    # Single PSUM pool, two size-classes: 4 x 1-bank slots + 2 x 2-bank
    # slots = all 8 banks. Every psum tile cycles through these slots.
    psp = tc.alloc_tile_pool(name="psp", bufs=1, space="PSUM")
    dramp = tc.alloc_tile_pool(name="dramp", bufs=1, space="DRAM")

    # ---- constants ----
    ident = consts.tile([P, P], BF16)
    make_identity(nc, ident)
    ones1 = consts.tile([1, P], BF16)
    nc.vector.memset(ones1, 1.0)
    eps_t = consts.tile([P, 1], F32)
    nc.vector.memset(eps_t, EPS)
    b2r = consts.tile([1, D], BF16)
    nc.sync.dma_start(b2r, io["b2r"])
    b1t = consts.tile([P, NF], F32)
    nc.sync.dma_start(b1t, io["b1t"])
    lnbc = {}
    for nm in ("ln1w", "ln1b", "ln2w", "ln2b"):
        t = consts.tile([P, D], F32, tag=nm, name=nm + "_bc")
        nc.gpsimd.dma_start(t, io[nm].broadcast_to([P, D]))
        lnbc[nm] = t

    spill = dramp.tile([T, D], F32, tag="spill", name="out1_spill")

    oTp = tc.alloc_tile_pool(name="oTp", bufs=1)
    oT = [oTp.tile([P, T], BF16, tag=f"oT{j}", name=f"oT{j}") for j in range(ND)]

    qkvp = tc.alloc_tile_pool(name="qkvp", bufs=1, side="right")
    qT = [qkvp.tile([P, T], BF16, tag=f"qT{j}", name=f"qT{j}") for j in range(ND)]
    kT = [qkvp.tile([P, T], BF16, tag=f"kT{j}", name=f"kT{j}") for j in range(ND)]
    # v tiles use a 65-column stride per head: 64 value columns + a ones
    # column, so the AV matmul also produces the softmax denominator row.
    VW = NHEAD * (HD + 1)
    vN = [qkvp.tile([P, VW], BF16, tag=f"v{i}", name=f"v{i}") for i in range(NT)]

    xtp = tc.alloc_tile_pool(name="xtp", bufs=1, side="right")
    wqkv = tc.alloc_tile_pool(name="wqkv", bufs=1, side="right")
    trA = tc.alloc_tile_pool(name="trA", bufs=1, side="right")

    # ---- weight loads for phase 1 ----
    wq_sb, wk_sb, wv_sb = [], [], []
    for k in range(ND):
        for lst, name in ((wq_sb, "wq"), (wk_sb, "wk"), (wv_sb, "wv")):
            t = wqkv.tile([P, D], BF16, tag=f"{name}{k}", name=f"{name}{k}_sb")
            nc.sync.dma_start(t, io[name][k * P:(k + 1) * P, :])
            lst.append(t)

    # ---- phase 0: load x, cast to bf16, build XT ----
    xt = [xtp.tile([P, T], BF16, tag=f"xt{j}", name=f"xt{j}") for j in range(ND)]
    for i in range(NT):
        x_t = trA.tile([P, D], F32, tag="xin", bufs=3, name="x_t")
        nc.sync.dma_start(x_t, io["x"][i * P:(i + 1) * P, :])
        x_bf = trA.tile([P, D], BF16, tag="xbf", bufs=2, name="x_bf")
        nc.gpsimd.tensor_copy(out=x_bf, in_=x_t)
        for j in range(ND):
            pt = psp.tile([P, P], BF16, tag="ps_small", bufs=4, name="pt")
            nc.tensor.transpose(pt, x_bf[:, j * P:(j + 1) * P], ident)
            nc.any.tensor_copy(out=xt[j][:, i * P:(i + 1) * P], in_=pt)

    # ---- phase 1: qT, kT (transposed), v (natural) ----
    for dst, w_sb in ((qT, wq_sb), (kT, wk_sb)):
        for m in range(ND):
            pss = [psp.tile([P, SEQ], F32, tag="ps_small", bufs=4, name=f"qk{g}")
                   for g in range(NB)]
            for k in range(ND):
                lhs = w_sb[k][:, m * P:(m + 1) * P]
                for g in range(NB):
                    nc.tensor.matmul(
                        pss[g], lhs, xt[k][:, g * SEQ:(g + 1) * SEQ],
                        start=(k == 0), stop=(k == ND - 1),
                    )
            for g in range(NB):
                nc.any.tensor_copy(
                    out=dst[m][:, g * SEQ:(g + 1) * SEQ], in_=pss[g])
    for i in range(NT):
        psv = psp.tile([P, D], F32, tag="ps_big", bufs=2, name="psv")
        for k in range(ND):
            lhs = xt[k][:, i * P:(i + 1) * P]
            nc.tensor.matmul(psv[:, :SEQ], lhs, wv_sb[k][:, :SEQ],
                             start=(k == 0), stop=(k == ND - 1))
            nc.tensor.matmul(psv[:, SEQ:D], lhs, wv_sb[k][:, SEQ:D],
                             start=(k == 0), stop=(k == ND - 1))
        nc.vector.memset(vN[i][:, HD::HD + 1], 1.0)
        for h in range(NHEAD):
            nc.any.tensor_copy(out=vN[i][:, h * (HD + 1):h * (HD + 1) + HD],
                               in_=psv[:, h * HD:(h + 1) * HD])

    trA.release()
    wqkv.release()
    xtp.release()

    # ---- wo loads emitted here: overlap with attention ----
    wop = tc.alloc_tile_pool(name="wop", bufs=1)
    wo_sb = []
    for k in range(ND):
        t = wop.tile([P, D], BF16, tag=f"wo{k}", name=f"wo{k}_sb")
        nc.sync.dma_start(t, io["wo"][k * P:(k + 1) * P, :])
        wo_sb.append(t)

    # ---- phase 2: attention, per (batch, head) ----
    ptp = tc.alloc_tile_pool(name="ptp", bufs=2, side="right")
    for b in range(NB):
        for h in range(NHEAD):
            mt = h // 2
            hoff = (h % 2) * HD
            q_h = qT[mt][hoff:hoff + HD, b * SEQ:(b + 1) * SEQ]
            pts = []
            for half in range(2):
                st = psp.tile([P, 2 * SEQ], F32, tag="ps_big", bufs=2, name="st")
                for t2 in range(2):
                    tchunk = half * 2 + t2
                    lhs = kT[mt][hoff:hoff + HD,
                                 b * SEQ + tchunk * P: b * SEQ + (tchunk + 1) * P]
                    nc.tensor.matmul(
                        st[:, t2 * SEQ:(t2 + 1) * SEQ], lhs, q_h,
                        start=True, stop=True)
                pt = ptp.tile([P, 2 * SEQ], BF16, tag=f"pt{half}", name=f"pt{half}")
                nc.scalar.activation(out=pt, in_=st, func=AF.Exp)
                pts.append(pt)
            ot = psp.tile([HD + 1, SEQ], F32, tag="ps_small", bufs=4, name="ot")
            for tchunk in range(4):
                vblk = vN[b * 4 + tchunk][:, h * (HD + 1):(h + 1) * (HD + 1)]
                nc.tensor.matmul(
                    ot, vblk,
                    pts[tchunk // 2][:, (tchunk % 2) * SEQ:(tchunk % 2 + 1) * SEQ],
                    start=(tchunk == 0), stop=(tchunk == 3))
            # row HD of ot = softmax denominator; normalize the head output
            rd = ptp.tile([1, SEQ], F32, tag="rd", name="rd")
            nc.vector.reciprocal(out=rd, in_=ot[HD:HD + 1, :])
            rdb = ptp.tile([1, SEQ], BF16, tag="rdb", name="rdb")
            nc.vector.tensor_copy(out=rdb, in_=rd)
            bc = psp.tile([HD, SEQ], F32, tag="ps_small", bufs=4, name="bc")
            nc.tensor.matmul(bc, ones1[0:1, 0:HD], rdb, start=True, stop=True)
            # TT may read only one PSUM operand -> stage bc in SBUF (bf16)
            bcs = ptp.tile([HD, SEQ], BF16, tag="bcs", name="bcs")
            nc.vector.tensor_copy(out=bcs, in_=bc)
            nc.vector.tensor_mul(
                out=oT[mt][hoff:hoff + HD, b * SEQ:(b + 1) * SEQ],
                in0=ot[0:HD, :], in1=bcs)
    ptp.release()
    qkvp.release()

    # ---- phase 3: mha = oT.T @ Wo ; out1 = x + LN1(mha) -> DRAM spill ----
    w1p = tc.alloc_tile_pool(name="w1p", bufs=1, side="right")
    w1_sb = [w1p.tile([P, FF], BF16, tag=f"w1_{k}", name=f"w1_{k}") for k in range(ND)]
    for k in range(ND):
        nc.sync.dma_start(w1_sb[k], io["w1"][k * P:(k + 1) * P, :])
    resid = tc.alloc_tile_pool(name="resid", bufs=1, side="right")
    o1T = [resid.tile([P, T], BF16, tag=f"o1T{j}", name=f"o1T{j}") for j in range(ND)]
    trB = tc.alloc_tile_pool(name="trB", bufs=2)

    for i in range(NT):
        mh = psp.tile([P, D], F32, tag="ps_big", bufs=2, name="mh")
        for k in range(ND):
            lhs = oT[k][:, i * P:(i + 1) * P]
            nc.tensor.matmul(mh[:, :SEQ], lhs, wo_sb[k][:, :SEQ],
                             start=(k == 0), stop=(k == ND - 1))
            nc.tensor.matmul(mh[:, SEQ:D], lhs, wo_sb[k][:, SEQ:D],
                             start=(k == 0), stop=(k == ND - 1))
        x_t = trB.tile([P, D], F32, tag="xin2", name="x_t2")
        nc.sync.dma_start(x_t, io["x"][i * P:(i + 1) * P, :])
        xb = trB.tile([P, D], F32, tag="xb", name="xb")
        nc.gpsimd.tensor_add(out=xb, in0=x_t, in1=lnbc["ln1b"])
        out1_t = trB.tile([P, D], F32, tag="out1", bufs=3, name="out1_t")
        _ln_apply(nc, trB, mh, lnbc["ln1w"], xb, out1_t, "1", eps_t)
        nc.sync.dma_start(spill[i * P:(i + 1) * P, :], out1_t)
        # bf16 copy + transpose for the FFN
        o1b = trB.tile([P, D], BF16, tag="o1b", name="o1b")
        nc.gpsimd.tensor_copy(out=o1b, in_=out1_t)
        for j in range(ND):
            pt = psp.tile([P, P], BF16, tag="ps_small", bufs=4, name="pt2")
            nc.tensor.transpose(pt, o1b[:, j * P:(j + 1) * P], ident)
            nc.any.tensor_copy(out=o1T[j][:, i * P:(i + 1) * P], in_=pt)

    trB.release()
    # w2 loads (overlap with the tail of phase 3 / start of FFN)
    w2p = tc.alloc_tile_pool(name="w2p", bufs=1, side="right")
    w2_sb = [w2p.tile([P, D], BF16, tag=f"w2_{k}", name=f"w2_{k}") for k in range(NF)]
    for k in range(NF):
        nc.sync.dma_start(w2_sb[k], io["w2"][k * P:(k + 1) * P, :])

    wop.release()
    oTp.release()

    # ---- phase 4: FFN + LN2 + final residual ----
    trC = tc.alloc_tile_pool(name="trC", bufs=2)
    hbuf = tc.alloc_tile_pool(name="hbuf", bufs=1)
    for g in range(NB):
        hts = []
        for f in range(NF):
            hp = psp.tile([P, SEQ], F32, tag="ps_small", bufs=4, name="hp")
            for k in range(ND):
                nc.tensor.matmul(
                    hp, w1_sb[k][:, f * P:(f + 1) * P],
                    o1T[k][:, g * SEQ:(g + 1) * SEQ],
                    start=(k == 0), stop=(k == ND - 1))
            ht = hbuf.tile([P, SEQ], BF16, tag=f"ht{f}", name=f"ht{f}")
            nc.scalar.activation(out=ht, in_=hp, func=AF.Gelu,
                                 bias=b1t[:, f:f + 1], scale=1.0)
            hts.append(ht)
        for sc in range(4):
            i = g * 4 + sc
            fp = psp.tile([P, D], F32, tag="ps_big", bufs=2, name="fp")
            for f in range(NF):
                lhs = hts[f][:, sc * P:(sc + 1) * P]
                nc.tensor.matmul(fp[:, :SEQ], lhs, w2_sb[f][:, :SEQ],
                                 start=(f == 0), stop=False)
                nc.tensor.matmul(fp[:, SEQ:D], lhs, w2_sb[f][:, SEQ:D],
                                 start=(f == 0), stop=False)
            # += b2 (rank-1), also closes both accum groups
            nc.tensor.matmul(fp[:, :SEQ], ones1, b2r[:, :SEQ],
                             start=False, stop=True)
            nc.tensor.matmul(fp[:, SEQ:D], ones1, b2r[:, SEQ:D],
                             start=False, stop=True)
            o1in = trC.tile([P, D], F32, tag="o1in", name="o1in")
            nc.sync.dma_start(o1in, spill[i * P:(i + 1) * P, :])
            base = trC.tile([P, D], F32, tag="base", name="base")
            nc.gpsimd.tensor_add(out=base, in0=o1in, in1=lnbc["ln2b"])
            outt = trC.tile([P, D], F32, tag="outt", name="outt")
            _ln_apply(nc, trC, fp, lnbc["ln2w"], base, outt, "2", eps_t)
            nc.sync.dma_start(io["out"][i * P:(i + 1) * P, :], outt)

    hbuf.release()
    trC.release()
    w2p.release()
    resid.release()
    w1p.release()
    consts.release()
    psp.release()
    dramp.release()


def build(bpc):
    """Build + compile the per-core program. Returns the Bacc object."""
    T = bpc * SEQ
    nc = bacc.Bacc("TRN2", target_bir_lowering=False, debug=False,
                   num_devices=N_CORES)
    io = {
        "x": nc.dram_tensor("x", [T, D], F32, kind="ExternalInput").ap(),
        "wq": nc.dram_tensor("wq", [D, D], BF16, kind="ExternalInput").ap(),
        "wk": nc.dram_tensor("wk", [D, D], BF16, kind="ExternalInput").ap(),
        "wv": nc.dram_tensor("wv", [D, D], BF16, kind="ExternalInput").ap(),
        "wo": nc.dram_tensor("wo", [D, D], BF16, kind="ExternalInput").ap(),
        "w1": nc.dram_tensor("w1", [D, FF], BF16, kind="ExternalInput").ap(),
        "w2": nc.dram_tensor("w2", [FF, D], BF16, kind="ExternalInput").ap(),
        "b1t": nc.dram_tensor("b1t", [P, NF], F32, kind="ExternalInput").ap(),
        "b2r": nc.dram_tensor("b2r", [1, D], BF16, kind="ExternalInput").ap(),
        "ln1w": nc.dram_tensor("ln1w", [1, D], F32, kind="ExternalInput").ap(),
        "ln1b": nc.dram_tensor("ln1b", [1, D], F32, kind="ExternalInput").ap(),
        "ln2w": nc.dram_tensor("ln2w", [1, D], F32, kind="ExternalInput").ap(),
        "ln2b": nc.dram_tensor("ln2b", [1, D], F32, kind="ExternalInput").ap(),
        "out": nc.dram_tensor("out", [T, D], F32, kind="ExternalOutput").ap(),
    }
    with tile.TileContext(nc) as tc:
        emit(nc, tc, io, bpc)
    nc.compile()
    return nc


def prep_weights(inputs):
    """Host-side weight layout prep (numpy only)."""
    bf = mybir.dt.np(BF16)
    f32 = np.float32
    wq = (np.asarray(inputs["Wq"], f32).transpose(1, 0, 2).reshape(D, D)
          / np.sqrt(HD)).astype(bf)
    wk = np.asarray(inputs["Wk"], f32).transpose(1, 0, 2).reshape(D, D).astype(bf)
    wv = np.asarray(inputs["Wv"], f32).transpose(1, 0, 2).reshape(D, D).astype(bf)
    return {
        "wq": np.ascontiguousarray(wq),
        "wk": np.ascontiguousarray(wk),
        "wv": np.ascontiguousarray(wv),
        "wo": np.asarray(inputs["Wo"], f32).astype(bf),
        "w1": np.asarray(inputs["W1"], f32).astype(bf),
        "w2": np.asarray(inputs["W2"], f32).astype(bf),
        "b1t": np.ascontiguousarray(
            np.asarray(inputs["b1"], f32).reshape(NF, P).T),
        "b2r": np.asarray(inputs["b2"], f32).reshape(1, D).astype(bf),
        "ln1w": np.asarray(inputs["ln1_w"], f32).reshape(1, D),
        "ln1b": np.asarray(inputs["ln1_b"], f32).reshape(1, D),
        "ln2w": np.asarray(inputs["ln2_w"], f32).reshape(1, D),
        "ln2b": np.asarray(inputs["ln2_b"], f32).reshape(1, D),
    }


_cache = {}


def kernel(**inputs) -> np.ndarray:
    bpc = B_TOTAL // N_CORES
    if "nc" not in _cache:
        _cache["nc"] = build(bpc)
    nc = _cache["nc"]
    w = prep_weights(inputs)
    x = np.asarray(inputs["x"], np.float32)
    in_maps = []
    for c in range(N_CORES):
        shard = np.ascontiguousarray(
            x[c * bpc:(c + 1) * bpc].reshape(bpc * SEQ, D))
        in_maps.append({"x": shard, **w})
    res = run_bass_kernel_spmd(nc, in_maps, list(range(N_CORES)))
    out = np.concatenate(
        [res.results[c]["out"].reshape(bpc, SEQ, D) for c in range(N_CORES)],
        axis=0)
    return np.ascontiguousarray(out.astype(np.float32))


# revision 16
# speedup vs baseline: 1.3546x; 1.3546x over previous
"""Trainium2 Bass kernel for a 1-layer transformer encoder.

Reference model (B=32, S=512, D=768, H=12, hd=64, hidden=3072):
    q,k,v = x@Wq, x@Wk, x@Wv         (per head)
    attn  = softmax(q k^T / 8) v
    mha   = concat_heads @ Wo
    out1  = x + LN(mha)
    ffn   = gelu(out1@W1 + b1) @ W2 + b2
    out   = out1 + LN(ffn)

Sharding: data-parallel over batch -- each of the 8 cores gets 4 full
sequences (2048 tokens) and all weights; no collectives.

Key kernel tricks:
  * All matmuls in bf16 (4x faster PE than fp32), fp32 PSUM accumulation.
  * Softmax computed as scores^T (t on partitions): exp without max
    subtraction (scores are O(1) here by construction) and WITHOUT the
    denominator: LN(c*v) == LN(v) for any per-token c>0, and the
    attention output only feeds Wo -> LN, so softmax normalization
    cancels mathematically.
  * LN rstd via ACT Sqrt + DVE reciprocal (production groupnorm idiom).
  * Per-feature LN scale/shift DMA-broadcast along partitions; b2 added
    via a rank-1 (ones x b2) matmul accumulated into PSUM.
  * Activations stay "feature-transposed" exactly where matmuls want
    them; only x and out1 need explicit PE transposes (96 blocks each).
"""

import os
import sys
from contextlib import ExitStack

import numpy as np

for _p in ("/opt/trn_rl_repo", "/root/.axon_site/_ro/trn_rl_repo"):
    if os.path.isdir(_p) and _p not in sys.path:
        sys.path.insert(0, _p)

os.environ.setdefault("MYCRO_LOCAL_CACHE", "1")

import concourse.bacc as bacc
import concourse.tile as tile
from concourse import mybir
from concourse.bass_utils import run_bass_kernel_spmd
from concourse.masks import make_identity

F32 = mybir.dt.float32
BF16 = mybir.dt.bfloat16
AF = mybir.ActivationFunctionType
OP = mybir.AluOpType

# model dims
D, NHEAD, HD, FF, SEQ, P = 768, 12, 64, 3072, 512, 128
ND = D // P    # 6 feature chunks
NF = FF // P   # 24 hidden chunks
EPS = 1e-5
N_CORES = 8
B_TOTAL = 32


def _ln_apply(nc, pool, src_psum, w_bc, b_plus_res, out_tile, tag, eps_tile):
    """out_tile = b_plus_res + ((src - mean)/std) * w_bc   (src: [128, 768] psum)."""
    stats = pool.tile([P, 3, 6], F32, tag=f"stats{tag}")
    for sg in range(3):
        nc.vector.bn_stats(out=stats[:, sg, :], in_=src_psum[:, sg * 256:(sg + 1) * 256])
    mv = pool.tile([P, 2], F32, tag=f"mv{tag}")
    nc.vector.bn_aggr(out=mv, in_=stats)
    rstd = pool.tile([P, 1], F32, tag=f"rstd{tag}")
    # rstd = 1/sqrt(var + eps)
    nc.scalar.activation(out=rstd, in_=mv[:, 1:2], func=AF.Sqrt, bias=eps_tile, scale=1.0)
    nc.vector.reciprocal(out=rstd, in_=rstd)
    # u = (src - mean) * w_bc   [one STT pass from PSUM]
    u = pool.tile([P, D], F32, tag=f"u{tag}")
    nc.vector.scalar_tensor_tensor(
        out=u, in0=src_psum, scalar=mv[:, 0:1], in1=w_bc,
        op0=OP.subtract, op1=OP.mult,
    )
    # out = u * rstd + b_plus_res
    nc.vector.scalar_tensor_tensor(
        out=out_tile, in0=u, scalar=rstd, in1=b_plus_res,
        op0=OP.mult, op1=OP.add,
    )


def emit(nc, tc, io, bpc):
    """Emit the whole encoder layer for one core holding `bpc` sequences."""
    T = bpc * SEQ
    NT = T // P          # token chunks
    NB = bpc             # 512-wide column groups == batches

    # Pools are opened/released manually: SBUF reservations are
    # [open, release] windows, and the phase structure is not LIFO.
    consts = tc.alloc_tile_pool(name="consts", bufs=1)
    # Single PSUM pool, two size-classes: 4 x 1-bank slots + 2 x 2-bank
    # slots = all 8 banks. Every psum tile cycles through these slots.
    psp = tc.alloc_tile_pool(name="psp", bufs=1, space="PSUM")
    dramp = tc.alloc_tile_pool(name="dramp", bufs=1, space="DRAM")

    # ---- constants ----
    ident = consts.tile([P, P], BF16)
    make_identity(nc, ident)
    ones1 = consts.tile([1, P], BF16)
    nc.vector.memset(ones1, 1.0)
    eps_t = consts.tile([P, 1], F32)
    nc.vector.memset(eps_t, EPS)
    b2r = consts.tile([1, D], BF16)
    nc.sync.dma_start(b2r, io["b2r"])
    b1t = consts.tile([P, NF], F32)
    nc.sync.dma_start(b1t, io["b1t"])
    lnbc = {}
    for nm in ("ln1w", "ln1b", "ln2w", "ln2b"):
        t = consts.tile([P, D], F32, tag=nm, name=nm + "_bc")
        nc.gpsimd.dma_start(t, io[nm].broadcast_to([P, D]))
        lnbc[nm] = t

    spill = dramp.tile([T, D], F32, tag="spill", name="out1_spill")

    oTp = tc.alloc_tile_pool(name="oTp", bufs=1)
    oT = [oTp.tile([P, T], BF16, tag=f"oT{j}", name=f"oT{j}") for j in range(ND)]

    qkvp = tc.alloc_tile_pool(name="qkvp", bufs=1, side="right")
    qT = [qkvp.tile([P, T], BF16, tag=f"qT{j}", name=f"qT{j}") for j in range(ND)]
    kT = [qkvp.tile([P, T], BF16, tag=f"kT{j}", name=f"kT{j}") for j in range(ND)]
    # v tiles use a 65-column stride per head: 64 value columns + a ones
    # column, so the AV matmul also produces the softmax denominator row.
    VW = NHEAD * (HD + 1)
    vN = [qkvp.tile([P, VW], BF16, tag=f"v{i}", name=f"v{i}") for i in range(NT)]

    xtp = tc.alloc_tile_pool(name="xtp", bufs=1, side="right")
    wqkv = tc.alloc_tile_pool(name="wqkv", bufs=1, side="right")
    trA = tc.alloc_tile_pool(name="trA", bufs=1, side="right")

    # ---- weight loads for phase 1 ----
    wq_sb, wk_sb, wv_sb = [], [], []
    for k in range(ND):
        for lst, name in ((wq_sb, "wq"), (wk_sb, "wk"), (wv_sb, "wv")):
            t = wqkv.tile([P, D], BF16, tag=f"{name}{k}", name=f"{name}{k}_sb")
            nc.sync.dma_start(t, io[name][k * P:(k + 1) * P, :])
            lst.append(t)

    # ---- phase 0: load x, cast to bf16, build XT ----
    xt = [xtp.tile([P, T], BF16, tag=f"xt{j}", name=f"xt{j}") for j in range(ND)]
    for i in range(NT):
        x_t = trA.tile([P, D], F32, tag="xin", bufs=3, name="x_t")
        nc.sync.dma_start(x_t, io["x"][i * P:(i + 1) * P, :])
        x_bf = trA.tile([P, D], BF16, tag="xbf", bufs=2, name="x_bf")
        nc.gpsimd.tensor_copy(out=x_bf, in_=x_t)
        for j in range(ND):
            pt = psp.tile([P, P], BF16, tag="ps_small", bufs=4, name="pt")
            nc.tensor.transpose(pt, x_bf[:, j * P:(j + 1) * P], ident)
            nc.any.tensor_copy(out=xt[j][:, i * P:(i + 1) * P], in_=pt)

    # ---- phase 1: qT, kT (transposed), v (natural) ----
    for dst, w_sb in ((qT, wq_sb), (kT, wk_sb)):
        for m in range(ND):
            pss = [psp.tile([P, SEQ], F32, tag="ps_small", bufs=4, name=f"qk{g}")
                   for g in range(NB)]
            for k in range(ND):
                lhs = w_sb[k][:, m * P:(m + 1) * P]
                for g in range(NB):
                    nc.tensor.matmul(
                        pss[g], lhs, xt[k][:, g * SEQ:(g + 1) * SEQ],
                        start=(k == 0), stop=(k == ND - 1),
                    )
            for g in range(NB):
                nc.any.tensor_copy(
                    out=dst[m][:, g * SEQ:(g + 1) * SEQ], in_=pss[g])
    for i in range(NT):
        psv = psp.tile([P, D], F32, tag="ps_big", bufs=2, name="psv")
        for k in range(ND):
            lhs = xt[k][:, i * P:(i + 1) * P]
            nc.tensor.matmul(psv[:, :SEQ], lhs, wv_sb[k][:, :SEQ],
                             start=(k == 0), stop=(k == ND - 1))
            nc.tensor.matmul(psv[:, SEQ:D], lhs, wv_sb[k][:, SEQ:D],
                             start=(k == 0), stop=(k == ND - 1))
        nc.vector.memset(vN[i][:, HD::HD + 1], 1.0)
        nc.any.tensor_copy(
            out=vN[i].rearrange("p (h w) -> p h w", w=HD + 1)[:, :, 0:HD],
            in_=psv.rearrange("p (h w) -> p h w", w=HD))

    trA.release()
    wqkv.release()
    xtp.release()

    # ---- wo loads emitted here: overlap with attention ----
    wop = tc.alloc_tile_pool(name="wop", bufs=1)
    wo_sb = []
    for k in range(ND):
        t = wop.tile([P, D], BF16, tag=f"wo{k}", name=f"wo{k}_sb")
        nc.sync.dma_start(t, io["wo"][k * P:(k + 1) * P, :])
        wo_sb.append(t)

    # ---- phase 2: attention, per (batch, head) ----
    ptp = tc.alloc_tile_pool(name="ptp", bufs=2, side="right")
    for b in range(NB):
        # raw (unnormalized) head outputs land in oT; denominator rows are
        # gathered per batch (via DMA: engine writes need 32-aligned start
        # partitions) so the reciprocal runs once on 12 lanes.
        denr = ptp.tile([NHEAD, SEQ], F32, tag="denr", name="denr")
        for h in range(NHEAD):
            mt = h // 2
            hoff = (h % 2) * HD
            q_h = qT[mt][hoff:hoff + HD, b * SEQ:(b + 1) * SEQ]
            pts = []
            for half in range(2):
                st = psp.tile([P, 2 * SEQ], F32, tag="ps_big", bufs=2, name="st")
                for t2 in range(2):
                    tchunk = half * 2 + t2
                    lhs = kT[mt][hoff:hoff + HD,
                                 b * SEQ + tchunk * P: b * SEQ + (tchunk + 1) * P]
                    nc.tensor.matmul(
                        st[:, t2 * SEQ:(t2 + 1) * SEQ], lhs, q_h,
                        start=True, stop=True)
                pt = ptp.tile([P, 2 * SEQ], BF16, tag=f"pt{half}", name=f"pt{half}")
                nc.scalar.activation(out=pt, in_=st, func=AF.Exp)
                pts.append(pt)
            ot = psp.tile([HD + 1, SEQ], F32, tag="ps_small", bufs=4, name="ot")
            for tchunk in range(4):
                vblk = vN[b * 4 + tchunk][:, h * (HD + 1):(h + 1) * (HD + 1)]
                nc.tensor.matmul(
                    ot, vblk,
                    pts[tchunk // 2][:, (tchunk % 2) * SEQ:(tchunk % 2 + 1) * SEQ],
                    start=(tchunk == 0), stop=(tchunk == 3))
            nc.vector.tensor_copy(
                out=oT[mt][hoff:hoff + HD, b * SEQ:(b + 1) * SEQ],
                in_=ot[0:HD, :])
            dstg = ptp.tile([1, SEQ], F32, tag="dstg", bufs=4, name="dstg")
            nc.vector.tensor_copy(out=dstg, in_=ot[HD:HD + 1, :])
            nc.sync.dma_start(denr[h:h + 1, :], dstg)
        rdv = ptp.tile([NHEAD, SEQ], F32, tag="rdv", name="rdv")
        nc.vector.reciprocal(out=rdv, in_=denr)
        rdb = ptp.tile([NHEAD, SEQ], BF16, tag="rdb", name="rdb")
        nc.vector.tensor_copy(out=rdb, in_=rdv)
        # bounce via DRAM: SBUF APs cannot broadcast along partitions
        rdb_d = dramp.tile([NHEAD, SEQ], BF16, tag="rdb_d", bufs=2, name="rdb_d")
        nc.sync.dma_start(rdb_d, rdb)
        for h in range(NHEAD):
            mt = h // 2
            hoff = (h % 2) * HD
            bcs = ptp.tile([P, SEQ], BF16, tag="bcs", name="bcs")
            nc.sync.dma_start(bcs[hoff:hoff + HD, :],
                              rdb_d[h:h + 1, :].broadcast_to([HD, SEQ]))
            sl = oT[mt][hoff:hoff + HD, b * SEQ:(b + 1) * SEQ]
            nc.vector.tensor_mul(out=sl, in0=sl, in1=bcs[hoff:hoff + HD, :])
    ptp.release()
    qkvp.release()

    # ---- phase 3: mha = oT.T @ Wo ; out1 = x + LN1(mha) -> DRAM spill ----
    w1p = tc.alloc_tile_pool(name="w1p", bufs=1, side="right")
    w1_sb = [w1p.tile([P, FF], BF16, tag=f"w1_{k}", name=f"w1_{k}") for k in range(ND)]
    for k in range(ND):
        nc.sync.dma_start(w1_sb[k], io["w1"][k * P:(k + 1) * P, :])
    resid = tc.alloc_tile_pool(name="resid", bufs=1, side="right")
    o1T = [resid.tile([P, T], BF16, tag=f"o1T{j}", name=f"o1T{j}") for j in range(ND)]
    trB = tc.alloc_tile_pool(name="trB", bufs=2)

    for i in range(NT):
        mh = psp.tile([P, D], F32, tag="ps_big", bufs=2, name="mh")
        for k in range(ND):
            lhs = oT[k][:, i * P:(i + 1) * P]
            nc.tensor.matmul(mh[:, :SEQ], lhs, wo_sb[k][:, :SEQ],
                             start=(k == 0), stop=(k == ND - 1))
            nc.tensor.matmul(mh[:, SEQ:D], lhs, wo_sb[k][:, SEQ:D],
                             start=(k == 0), stop=(k == ND - 1))
        x_t = trB.tile([P, D], F32, tag="xin2", name="x_t2")
        nc.sync.dma_start(x_t, io["x"][i * P:(i + 1) * P, :])
        xb = trB.tile([P, D], F32, tag="xb", name="xb")
        nc.gpsimd.tensor_add(out=xb, in0=x_t, in1=lnbc["ln1b"])
        out1_t = trB.tile([P, D], F32, tag="out1", bufs=3, name="out1_t")
        _ln_apply(nc, trB, mh, lnbc["ln1w"], xb, out1_t, "1", eps_t)
        nc.sync.dma_start(spill[i * P:(i + 1) * P, :], out1_t)
        # bf16 copy + transpose for the FFN
        o1b = trB.tile([P, D], BF16, tag="o1b", name="o1b")
        nc.scalar.copy(out=o1b, in_=out1_t)
        for j in range(ND):
            pt = psp.tile([P, P], BF16, tag="ps_small", bufs=4, name="pt2")
            nc.tensor.transpose(pt, o1b[:, j * P:(j + 1) * P], ident)
            nc.any.tensor_copy(out=o1T[j][:, i * P:(i + 1) * P], in_=pt)

    trB.release()
    # w2 loads (overlap with the tail of phase 3 / start of FFN)
    w2p = tc.alloc_tile_pool(name="w2p", bufs=1, side="right")
    w2_sb = [w2p.tile([P, D], BF16, tag=f"w2_{k}", name=f"w2_{k}") for k in range(NF)]
    for k in range(NF):
        nc.sync.dma_start(w2_sb[k], io["w2"][k * P:(k + 1) * P, :])

    wop.release()
    oTp.release()

    # ---- phase 4: FFN + LN2 + final residual ----
    trC = tc.alloc_tile_pool(name="trC", bufs=2)
    hbuf = tc.alloc_tile_pool(name="hbuf", bufs=1)
    for g in range(NB):
        hts = []
        for f in range(NF):
            hp = psp.tile([P, SEQ], F32, tag="ps_small", bufs=4, name="hp")
            for k in range(ND):
                nc.tensor.matmul(
                    hp, w1_sb[k][:, f * P:(f + 1) * P],
                    o1T[k][:, g * SEQ:(g + 1) * SEQ],
                    start=(k == 0), stop=(k == ND - 1))
            ht = hbuf.tile([P, SEQ], BF16, tag=f"ht{f}", name=f"ht{f}")
            nc.scalar.activation(out=ht, in_=hp, func=AF.Gelu,
                                 bias=b1t[:, f:f + 1], scale=1.0)
            hts.append(ht)
        for sc in range(4):
            i = g * 4 + sc
            fp = psp.tile([P, D], F32, tag="ps_big", bufs=2, name="fp")
            for f in range(NF):
                lhs = hts[f][:, sc * P:(sc + 1) * P]
                nc.tensor.matmul(fp[:, :SEQ], lhs, w2_sb[f][:, :SEQ],
                                 start=(f == 0), stop=False)
                nc.tensor.matmul(fp[:, SEQ:D], lhs, w2_sb[f][:, SEQ:D],
                                 start=(f == 0), stop=False)
            # += b2 (rank-1), also closes both accum groups
            nc.tensor.matmul(fp[:, :SEQ], ones1, b2r[:, :SEQ],
                             start=False, stop=True)
            nc.tensor.matmul(fp[:, SEQ:D], ones1, b2r[:, SEQ:D],
                             start=False, stop=True)
            o1in = trC.tile([P, D], F32, tag="o1in", name="o1in")
            nc.sync.dma_start(o1in, spill[i * P:(i + 1) * P, :])
            base = trC.tile([P, D], F32, tag="base", name="base")
            nc.gpsimd.tensor_add(out=base, in0=o1in, in1=lnbc["ln2b"])
            outt = trC.tile([P, D], F32, tag="outt", name="outt")
            _ln_apply(nc, trC, fp, lnbc["ln2w"], base, outt, "2", eps_t)
            nc.sync.dma_start(io["out"][i * P:(i + 1) * P, :], outt)

    hbuf.release()
    trC.release()
    w2p.release()
    resid.release()
    w1p.release()
    consts.release()
    psp.release()
    dramp.release()


def build(bpc):
    """Build + compile the per-core program. Returns the Bacc object."""
    T = bpc * SEQ
    nc = bacc.Bacc("TRN2", target_bir_lowering=False, debug=False,
                   num_devices=N_CORES)
    io = {
        "x": nc.dram_tensor("x", [T, D], F32, kind="ExternalInput").ap(),
        "wq": nc.dram_tensor("wq", [D, D], BF16, kind="ExternalInput").ap(),
        "wk": nc.dram_tensor("wk", [D, D], BF16, kind="ExternalInput").ap(),
        "wv": nc.dram_tensor("wv", [D, D], BF16, kind="ExternalInput").ap(),
        "wo": nc.dram_tensor("wo", [D, D], BF16, kind="ExternalInput").ap(),
        "w1": nc.dram_tensor("w1", [D, FF], BF16, kind="ExternalInput").ap(),
        "w2": nc.dram_tensor("w2", [FF, D], BF16, kind="ExternalInput").ap(),
        "b1t": nc.dram_tensor("b1t", [P, NF], F32, kind="ExternalInput").ap(),
        "b2r": nc.dram_tensor("b2r", [1, D], BF16, kind="ExternalInput").ap(),
        "ln1w": nc.dram_tensor("ln1w", [1, D], F32, kind="ExternalInput").ap(),
        "ln1b": nc.dram_tensor("ln1b", [1, D], F32, kind="ExternalInput").ap(),
        "ln2w": nc.dram_tensor("ln2w", [1, D], F32, kind="ExternalInput").ap(),
        "ln2b": nc.dram_tensor("ln2b", [1, D], F32, kind="ExternalInput").ap(),
        "out": nc.dram_tensor("out", [T, D], F32, kind="ExternalOutput").ap(),
    }
    with tile.TileContext(nc) as tc:
        emit(nc, tc, io, bpc)
    nc.compile()
    return nc


def prep_weights(inputs):
    """Host-side weight layout prep (numpy only)."""
    bf = mybir.dt.np(BF16)
    f32 = np.float32
    wq = (np.asarray(inputs["Wq"], f32).transpose(1, 0, 2).reshape(D, D)
          / np.sqrt(HD)).astype(bf)
    wk = np.asarray(inputs["Wk"], f32).transpose(1, 0, 2).reshape(D, D).astype(bf)
    wv = np.asarray(inputs["Wv"], f32).transpose(1, 0, 2).reshape(D, D).astype(bf)
    return {
        "wq": np.ascontiguousarray(wq),
        "wk": np.ascontiguousarray(wk),
        "wv": np.ascontiguousarray(wv),
        "wo": np.asarray(inputs["Wo"], f32).astype(bf),
        "w1": np.asarray(inputs["W1"], f32).astype(bf),
        "w2": np.asarray(inputs["W2"], f32).astype(bf),
        "b1t": np.ascontiguousarray(
            np.asarray(inputs["b1"], f32).reshape(NF, P).T),
        "b2r": np.asarray(inputs["b2"], f32).reshape(1, D).astype(bf),
        "ln1w": np.asarray(inputs["ln1_w"], f32).reshape(1, D),
        "ln1b": np.asarray(inputs["ln1_b"], f32).reshape(1, D),
        "ln2w": np.asarray(inputs["ln2_w"], f32).reshape(1, D),
        "ln2b": np.asarray(inputs["ln2_b"], f32).reshape(1, D),
    }


_cache = {}


def kernel(**inputs) -> np.ndarray:
    bpc = B_TOTAL // N_CORES
    if "nc" not in _cache:
        _cache["nc"] = build(bpc)
    nc = _cache["nc"]
    w = prep_weights(inputs)
    x = np.asarray(inputs["x"], np.float32)
    in_maps = []
    for c in range(N_CORES):
        shard = np.ascontiguousarray(
            x[c * bpc:(c + 1) * bpc].reshape(bpc * SEQ, D))
        in_maps.append({"x": shard, **w})
    res = run_bass_kernel_spmd(nc, in_maps, list(range(N_CORES)))
    out = np.concatenate(
        [res.results[c]["out"].reshape(bpc, SEQ, D) for c in range(N_CORES)],
        axis=0)
    return np.ascontiguousarray(out.astype(np.float32))
